# revision 18
# baseline (speedup 1.0000x reference)
"""Trainium2 8-core kernel for nn_AttModule (sparse sliding-window attention).

Sequence-parallel: L=131072 split into 8 shards of 16384. Halos staged host-side
(conv needs +-96 of x, attention windows need +-32 of k/v context). The only
on-device collective is a 2KB AllReduce of InstanceNorm sum/sumsq.

v6 design notes (vs the 814us baseline):
 - Every InstMatmult self-loads its stationary (no LDWEIGHTS elision), so the
   wins come from FEWER matmuls:
   * vT is not built from 514 f-stationary matmuls: v = Wv@f with 33 wide
     fp8-DoubleRow matmuls (f and Wv staged fp8e4), then DMA XBAR transposes
     (SBUF->SBUF) give vt / vt_o tiles for the ov stationaries. The v chunks
     and transposes are interleaved INTO the conv flights; transposes must
     not overlap the AllReduce (observed data corruption), so everything v
     finishes before the stats collective.
   * conv / k / q / Wc stay bf16: fp8 anywhere on the conv-out -> h residual
     path blows the 2e-2 budget (measured 2.4-3.2e-2).
 - Attention is software-pipelined as front_a(g+1) | front_b(g) | back(g-1):
   the PE FIFO holds the next group's k/q/energy matmuls while group g's
   softmax chain (ones-matmul denominator -> 3.2us DVE reciprocal -> gpsimd
   normalize) drains; back() then finds patt ready. Without the skew the PE
   head-of-line blocks ~3.9us per group.
 - Engine balance: ACT owns exp + all psum+bias evacuations (Identity/Relu
   are tableless; only Exp's table stays hot - Ln/Exp alternation would
   reload 1.3us tables per group), DVE owns mask-mul/reciprocal/h-residual,
   GPSIMD owns the normalize-mul and y residual adds.
 - Per-flight ACT Square stats sweeps (s2) run inside the conv phase; only
   the 2KB stats AllReduce separates conv from attention, with dummy warm
   matmuls keeping the PE activity monitor at full clock through it.
 - InstanceNorm folded into wq/wk on device; 1/sqrt(CQ) applied as the ACT
   Exp scale; bv folded post-ov (att rows sum to one); y written bf16 and
   upcast host-side; residual x staged bf16.
"""

import os
import sys

import numpy as np

try:
    import concourse.bass as bass  # noqa: F401
except ImportError:
    sys.path.insert(0, "/opt/trn_rl_repo")

import concourse.bacc as bacc
import concourse.bass as bass
import concourse.mybir as mybir
import concourse.tile as tile
from concourse.bass_utils import run_bass_kernel_spmd

import ml_dtypes

BF16 = ml_dtypes.bfloat16
FP8 = ml_dtypes.float8_e4m3fn

N_CORES = 8
C = 256
P = 128
CQ = 128
CV = 128
BL = 64
HALF = 32
L = 131072
LLOC = L // N_CORES              # 16384
EXT = LLOC + 2 * HALF            # 16448 conv-out/k region (+-32 halo)
NVT = EXT // P + 1               # 129 even vT tiles
NVO = LLOC // P                  # 128 odd vT tiles
FW = NVT * P                     # 16512 staged f width
XW = LLOC + 2 * (BL + HALF)      # 16576 staged x width (+-96 halo)
NB = LLOC // BL                  # 256 blocks per core
GB = 8                           # blocks per group
NG = NB // GB                    # 32 groups
GW = GB * BL                     # 512 positions per group
EPS_IN = 1e-5
ESCALE = 1.0 / float(np.sqrt(CQ))

FP32 = mybir.dt.float32
BF = mybir.dt.bfloat16
F8 = mybir.dt.float8e4
AF = mybir.ActivationFunctionType
ALU = mybir.AluOpType
DR = mybir.MatmulPerfMode.DoubleRow

_CACHE = {}


def _build_graph():
    nc = bacc.Bacc(None, target_bir_lowering=False, debug=False)

    ext_in = {}
    for name, shape, dt in [
        ("xb", [P, 2, XW], BF),
        ("xr", [C, LLOC], BF),
        ("fb", [P, 2, FW], F8),
        ("wff", [P, 1536], BF),
        ("wq", [P, 2, P], BF),
        ("wk", [P, 2, P], BF),
        ("wvd", [P, 2, P], F8),
        ("wo", [P, 256], BF),
        ("wc", [P, 512], BF),
        ("bias", [P, 9], FP32),
        ("fm", [P, 1536], BF),
        ("ones", [P, P], BF),
    ]:
        ext_in[name] = nc.declare_dram_parameter(name, shape, dt, isOutput=False)
    y_ext = nc.declare_dram_parameter("y", [C, LLOC], BF, isOutput=True)

    with tile.TileContext(nc) as tc:
        with (
            tc.tile_pool(name="const", bufs=1) as constp,
            tc.tile_pool(name="big", bufs=1) as bigp,
            tc.tile_pool(name="dram", bufs=1, space="DRAM") as dramp,
        ):
            # ---- constants to SBUF ----
            wff = constp.tile([P, 1536], BF, tag="wff")
            nc.sync.dma_start(wff[:], ext_in["wff"][:])
            wq = constp.tile([P, 2, P], BF, tag="wq")
            nc.sync.dma_start(wq[:], ext_in["wq"][:])
            wk = constp.tile([P, 2, P], BF, tag="wk")
            nc.sync.dma_start(wk[:], ext_in["wk"][:])
            wvd = constp.tile([P, 2, P], F8, tag="wvd")
            nc.sync.dma_start(wvd[:], ext_in["wvd"][:])
            wo = constp.tile([P, 256], BF, tag="wo")
            nc.sync.dma_start(wo[:], ext_in["wo"][:])
            wc = constp.tile([P, 512], BF, tag="wc")
            nc.sync.dma_start(wc[:], ext_in["wc"][:])
            bias = constp.tile([P, 9], FP32, tag="bias")
            nc.sync.dma_start(bias[:], ext_in["bias"][:])
            fm = constp.tile([P, 1536], BF, tag="fm")
            nc.sync.dma_start(fm[:], ext_in["fm"][:])
            ones = constp.tile([P, P], BF, tag="ones")
            nc.sync.dma_start(ones[:], ext_in["ones"][:])
            zeros = constp.tile([P, 512], BF, tag="zeros")
            nc.vector.memset(zeros[:], 0.0)

            # ---- persistent big tensors ----
            out_e = bigp.tile([P, 2, EXT], BF, tag="out_e")
            vt = bigp.tile([P, NVT, P], BF, tag="vt")
            vt_o = bigp.tile([P, NVO, P], BF, tag="vt_o")

            s1p = [constp.tile([P, NG], FP32, tag=f"s1p{h}", name=f"s1p{h}")
                   for h in range(2)]
            s2p = [constp.tile([P, NG], FP32, tag=f"s2p{h}", name=f"s2p{h}")
                   for h in range(2)]

            stats_in = dramp.tile([C, 2], FP32)
            stats_out = dramp.tile([C, 2], FP32)

            # ---- phase 1: dilated conv + ReLU (bf16), v/vT interleaved ----
            # col-groups over ext cols: [0,32) | 32 x 512 | [16416,16448)
            conv_groups = [(0, 32, None)] + [
                (32 + g * 512, 512, g) for g in range(NG)
            ] + [(EXT - 32, 32, None)]
            flights = [conv_groups[i:i + 3] for i in range(0, 34, 3)]
            # v chunks (33 of 512) spread over the flights; vdr writes +
            # transposes chase the chunks.
            with (
                tc.tile_pool(name="xs", bufs=3) as xsp,
                tc.tile_pool(name="fs", bufs=4) as fsp,
                tc.tile_pool(name="vsb", bufs=1) as vsbp,
                tc.tile_pool(name="cps", bufs=1, space="PSUM") as cps,
                tc.tile_pool(name="vps", bufs=2, space="PSUM") as vps,
            ):
                v_sb = vsbp.tile([P, FW], BF, tag="v_sb")

                def v_chunk(j):
                    w = 512 if j < 32 else FW - 32 * 512
                    f_t = fsp.tile([P, 2, 512], F8, tag="f_t")
                    nc.sync.dma_start(
                        f_t[:, :, :w], ext_in["fb"][:, :, j * 512:j * 512 + w])
                    psv = vps.tile([P, 512], FP32, tag="psv")
                    nc.tensor.matmul(
                        psv[:, :w], wvd[:], f_t[:, :, :w],
                        start=True, stop=True, perf_mode=DR,
                    )
                    nc.scalar.activation(
                        v_sb[:, j * 512:j * 512 + w], psv[:, :w], AF.Copy)

                def v_transpose(c):
                    # c in [0, 16): even -> vt chunk, odd -> vt_o chunk.
                    # SBUF-source XBAR transpose: the tile framework tracks
                    # the v_sb RAW dependency (a DRAM bounce does not).
                    eng = nc.scalar if c % 2 == 0 else nc.sync
                    t0 = (c // 2) * 16
                    if c % 2 == 0:
                        ntl = 16 if c // 2 < 7 else NVT - 16 * 7
                        eng.dma_start(
                            vt[:, t0:t0 + ntl, :],
                            v_sb[:, t0 * P:(t0 + ntl) * P], transpose=True)
                    else:
                        eng.dma_start(
                            vt_o[:, t0:t0 + 16, :],
                            v_sb[:, 64 + t0 * P:64 + (t0 + 16) * P],
                            transpose=True)

                vj = 0
                vc = 0
                for fi, flight in enumerate(flights):
                    a0 = flight[0][0]
                    a1 = flight[-1][0] + flight[-1][1]
                    span = a1 - a0 + 128
                    xh = xsp.tile([P, 2, 1664], BF, tag="xh")
                    qs = span // 4
                    cuts = [0, qs, 2 * qs, 3 * qs, span]
                    for ci in range(4):
                        eng = nc.sync if ci % 2 == 0 else nc.scalar
                        for h in range(2):
                            eng.dma_start(
                                xh[:, h, cuts[ci]:cuts[ci + 1]],
                                ext_in["xb"][:, h, a0 + cuts[ci]:
                                             a0 + cuts[ci + 1]])
                    ps = {}
                    for gi, (a, n, sg) in enumerate(flight):
                        for o in range(2):
                            ps[(a, o)] = cps.tile(
                                [P, 512], FP32, tag=f"cps{gi}{o}",
                                name=f"cps{gi}{o}")
                    for o in range(2):
                        for i in range(2):
                            for tap in range(3):
                                w = wff[:, ((tap * 2 + i) * 2 + o) * P:
                                        ((tap * 2 + i) * 2 + o + 1) * P]
                                for (a, n, sg) in flight:
                                    nc.tensor.matmul(
                                        ps[(a, o)][:, :n], w,
                                        xh[:, i, (a - a0) + tap * 64:
                                           (a - a0) + tap * 64 + n],
                                        start=(i == 0 and tap == 0),
                                        stop=(i == 1 and tap == 2),
                                        skip_group_check=True,
                                    )
                    for (a, n, sg) in flight:
                        for o in range(2):
                            if sg is not None:
                                nc.scalar.activation(
                                    out_e[:, o, a:a + n], ps[(a, o)][:, :n],
                                    AF.Relu, bias=bias[:, o:o + 1],
                                    accum_out=s1p[o][:, sg:sg + 1],
                                )
                                nc.scalar.activation(
                                    zeros[:], out_e[:, o, a:a + n],
                                    AF.Square, accum_out=s2p[o][:, sg:sg + 1],
                                )
                            else:
                                nc.scalar.activation(
                                    out_e[:, o, a:a + n], ps[(a, o)][:, :n],
                                    AF.Relu, bias=bias[:, o:o + 1],
                                )
                    # chase with v chunks / transposes
                    for _ in range(3):
                        if vj < 33:
                            v_chunk(vj)
                            vj += 1
                    while vc < 16 and (vc // 2 + 1) * 16 * P + 64 <= vj * 512:
                        v_transpose(vc)
                        vc += 1
                while vj < 33:
                    v_chunk(vj)
                    vj += 1
                while vc < 16:
                    v_transpose(vc)
                    vc += 1
                # stats reduce + DMA out, then dummy matmuls keep the PE
                # array's activity monitor at full clock through the
                # AllReduce latency (real work resumes at the fold)
                for h in range(2):
                    st = constp.tile([P, 2], FP32, tag=f"st{h}", name=f"st{h}")
                    nc.vector.tensor_reduce(
                        st[:, 0:1], s1p[h][:], mybir.AxisListType.X, ALU.add)
                    nc.vector.tensor_reduce(
                        st[:, 1:2], s2p[h][:], mybir.AxisListType.X, ALU.add)
                    nc.sync.dma_start(stats_in[h * P:(h + 1) * P, :], st[:])
                for dmy in range(48):
                    dps = vps.tile([P, 512], FP32, tag="psv", name="dmy")
                    nc.tensor.matmul(
                        dps[:], ones[:], zeros[:], start=True, stop=True,
                        skip_group_check=True)

            # ---- AllReduce (stats were reduced + written during conv) ----
            nc.gpsimd.collective_compute(
                "AllReduce", ALU.add,
                replica_groups=[list(range(N_CORES))],
                ins=[stats_in.opt()],
                outs=[stats_out.opt()],
            )

            # ---- phase 2: stats -> mu, rstd; fold norm into wq/wk ----
            with tc.tile_pool(name="psf", bufs=2, space="PSUM") as psf:
                sb = []
                for h in range(2):
                    s = constp.tile([P, 2], FP32, tag=f"sb{h}", name=f"sb{h}")
                    nc.sync.dma_start(s[:], stats_out[h * P:(h + 1) * P, :])
                    sb.append(s)
                wq_e = constp.tile([P, 2, P], BF, tag="wq_e")
                wk_e = constp.tile([P, 2, P], BF, tag="wk_e")
                bq_e = constp.tile([P, 1], FP32, tag="bq_e")
                bk_e = constp.tile([P, 1], FP32, tag="bk_e")
                mu_bf = []
                for h in range(2):
                    mu = constp.tile([P, 1], FP32, tag=f"mu{h}", name=f"mu{h}")
                    nc.vector.tensor_scalar_mul(mu[:], sb[h][:, 0:1], 1.0 / L)
                    ex2 = constp.tile([P, 1], FP32, tag=f"ex2{h}", name=f"ex2{h}")
                    nc.vector.tensor_scalar_mul(ex2[:], sb[h][:, 1:2], 1.0 / L)
                    mu2 = constp.tile([P, 1], FP32, tag=f"mu2{h}", name=f"mu2{h}")
                    nc.vector.tensor_mul(mu2[:], mu[:], mu[:])
                    var = constp.tile([P, 1], FP32, tag=f"var{h}", name=f"var{h}")
                    nc.vector.tensor_sub(var[:], ex2[:], mu2[:])
                    nc.vector.tensor_scalar_add(var[:], var[:], float(EPS_IN))
                    sd = constp.tile([P, 1], FP32, tag=f"sd{h}", name=f"sd{h}")
                    nc.scalar.activation(sd[:], var[:], AF.Sqrt)
                    rs = constp.tile([P, 1], FP32, tag=f"rs{h}", name=f"rs{h}")
                    nc.vector.reciprocal(rs[:], sd[:])
                    mb = constp.tile([P, 1], BF, tag=f"mub{h}", name=f"mub{h}")
                    nc.vector.tensor_copy(mb[:], mu[:])
                    mu_bf.append(mb)
                    nc.vector.tensor_scalar_mul(
                        wq_e[:, h, :], wq[:, h, :], rs[:])
                    nc.vector.tensor_scalar_mul(
                        wk_e[:, h, :], wk[:, h, :], rs[:])
                for w_e, b_col, b_out in ((wq_e, 2, bq_e), (wk_e, 3, bk_e)):
                    psb = psf.tile([P, 512], FP32, tag="psb")
                    for h in range(2):
                        nc.tensor.matmul(
                            psb[:, 0:1], w_e[:, h, :], mu_bf[h][:],
                            start=(h == 0), stop=(h == 1),
                        )
                    nc.vector.tensor_sub(
                        b_out[:], bias[:, b_col:b_col + 1], psb[:, 0:1])

            # ---- phase 3: attention + output, software-pipelined ----
            # front_a(g): k/q -> energy -> exp/mask; front_b(g): denominator
            # matmul -> DVE reciprocal -> gpsimd normalize; back(g): ov -> Wo
            # -> Wc -> y. Emission order front_a(g+1), front_b(g), back(g-1)
            # keeps ~5us of independent matmuls queued on the PE while group
            # g's softmax reciprocal chain drains on DVE/GPSIMD.
            with (
                tc.tile_pool(name="kq", bufs=4) as kqp,
                tc.tile_pool(name="att", bufs=5) as attp,
                tc.tile_pool(name="hb", bufs=3) as hbp,
                tc.tile_pool(name="xr", bufs=2) as xrp,
                tc.tile_pool(name="ys", bufs=2) as ysp,
                tc.tile_pool(name="ps", bufs=1, space="PSUM") as psp,
                tc.tile_pool(name="psd", bufs=2, space="PSUM") as psdp,
            ):
                state = {}

                def front_a(g):
                    st = {}
                    if g % 2 == 0:
                        xr_t = xrp.tile([P, 2, 1024], BF, tag="xr")
                        for o in range(2):
                            nc.scalar.dma_start(
                                xr_t[:, o, :],
                                ext_in["xr"][o * P:(o + 1) * P,
                                             g * GW:g * GW + 1024])
                        st["xr_t"] = xr_t
                        st["y_t"] = ysp.tile([P, 2, 1024], BF, tag="y_t",
                                             name="y_t")
                    else:
                        st["xr_t"] = state[g - 1]["xr_t"]
                        st["y_t"] = state[g - 1]["y_t"]

                    # k for ext cols [g*512, g*512+640)
                    kw = 640 if g < NG - 1 else 576
                    k_g = kqp.tile([P, 640], BF, tag="kg")
                    kps = psp.tile([P, 1024], FP32, tag="kps")
                    for h in range(2):
                        nc.tensor.matmul(
                            kps[:, :512], wk_e[:, h, :],
                            out_e[:, h, g * GW:g * GW + 512],
                            start=(h == 0), stop=(h == 1),
                        )
                        nc.tensor.matmul(
                            kps[:, 512:kw], wk_e[:, h, :],
                            out_e[:, h, g * GW + 512:g * GW + kw],
                            start=(h == 0), stop=(h == 1),
                            skip_group_check=True,
                        )
                    nc.scalar.activation(k_g[:, :kw], kps[:, :kw],
                                         AF.Identity, bias=bk_e[:])
                    # q for this group's 512 own positions
                    qps = psp.tile([P, 512], FP32, tag="qps")
                    for h in range(2):
                        nc.tensor.matmul(
                            qps[:], wq_e[:, h, :],
                            out_e[:, h, HALF + g * GW:HALF + (g + 1) * GW],
                            start=(h == 0), stop=(h == 1),
                        )
                    q_t = kqp.tile([P, GW], BF, tag="q")
                    nc.scalar.activation(q_t[:], qps[:], AF.Identity,
                                         bias=bq_e[:])

                    # energy, transposed: pe[key, query] per 64-q block
                    pe = psp.tile([P, GW], FP32, tag="pe")
                    for b in range(GB):
                        nc.tensor.matmul(
                            pe[:, b * BL:(b + 1) * BL],
                            k_g[:, b * BL:b * BL + 2 * BL],
                            q_t[:, b * BL:(b + 1) * BL],
                            start=(b == 0), stop=(b == GB - 1),
                            skip_group_check=True,
                        )
                    pt = attp.tile([P, GW], BF, tag="pt")
                    nc.scalar.activation(pt[:], pe[:], AF.Exp, scale=ESCALE)
                    if g == 0:
                        fcol = 512
                    elif g == NG - 1:
                        fcol = 1024
                    else:
                        fcol = 0
                    pts = attp.tile([P, GW], BF, tag="pts")
                    nc.vector.tensor_mul(pts[:], pt[:], fm[:, fcol:fcol + GW])
                    st["pts"] = pts
                    state[g] = st

                def front_b(g):
                    st = state[g]
                    pts = st["pts"]
                    # denominator (ones matmul broadcasts across partitions)
                    pd = psdp.tile([P, GW], FP32, tag="pd")
                    nc.tensor.matmul(pd[:], ones[:], pts[:], start=True,
                                     stop=True)
                    rbc = attp.tile([P, GW], BF, tag="rbc")
                    with nc.allow_low_precision(reason="softmax recip bf16"):
                        nc.vector.reciprocal(rbc[:], pd[:])
                    patt = attp.tile([P, GW], BF, tag="patt")
                    nc.gpsimd.tensor_mul(patt[:], pts[:], rbc[:])
                    st["patt"] = patt  # noqa

                def back(g):
                    st = state.pop(g)
                    patt = st["patt"]
                    xr_t = st["xr_t"]
                    y_t = st["y_t"]
                    gc = (g % 2) * GW
                    # ov: 8 blocks into one psum bank -> one relu+bv
                    po = psp.tile([P, GW], FP32, tag="po")
                    for b in range(GB):
                        B = g * GB + b
                        if B % 2 == 0:
                            lhs = vt[:, B // 2, :]
                        else:
                            lhs = vt_o[:, (B - 1) // 2, :]
                        nc.tensor.matmul(
                            po[:, b * BL:(b + 1) * BL], lhs,
                            patt[:, b * BL:(b + 1) * BL],
                            start=(b == 0), stop=(b == GB - 1),
                            skip_group_check=True,
                        )
                    rov = attp.tile([P, GW], BF, tag="rov")
                    nc.scalar.activation(rov[:], po[:], AF.Relu,
                                         bias=bias[:, 4:5])

                    # Wo + residual with conv out -> h ; Wc + bc + x -> y
                    h_t = hbp.tile([P, 2, GW], BF, tag="h_t")
                    for o in range(2):
                        wps = psp.tile([P, 512], FP32, tag="wps",
                                       name=f"wo_{o}")
                        nc.tensor.matmul(
                            wps[:], wo[:, o * P:(o + 1) * P], rov[:],
                            start=True, stop=True)
                        nc.vector.scalar_tensor_tensor(
                            h_t[:, o, :], wps[:], bias[:, 5 + o:6 + o],
                            out_e[:, o, HALF + g * GW:HALF + (g + 1) * GW],
                            ALU.add, ALU.add)
                    for o in range(2):
                        wps = psp.tile([P, 512], FP32, tag="wps",
                                       name=f"wc_{o}")
                        for i in range(2):
                            nc.tensor.matmul(
                                wps[:],
                                wc[:, (i * 2 + o) * P:(i * 2 + o + 1) * P],
                                h_t[:, i, :],
                                start=(i == 0), stop=(i == 1),
                            )
                        yb = attp.tile([P, GW], BF, tag="yb",
                                       name=f"yb{o}")
                        nc.scalar.activation(yb[:], wps[:], AF.Identity,
                                             bias=bias[:, 7 + o:8 + o])
                        nc.gpsimd.tensor_add(
                            y_t[:, o, gc:gc + GW], yb[:],
                            xr_t[:, o, gc:gc + GW])
                    if g % 2 == 1:
                        for o in range(2):
                            nc.sync.dma_start(
                                y_ext[o * P:(o + 1) * P,
                                      (g - 1) * GW:(g + 1) * GW],
                                y_t[:, o, :])

                front_a(0)
                front_a(1)
                front_b(0)
                for g in range(1, NG):
                    if g + 1 < NG:
                        front_a(g + 1)
                    front_b(g)
                    back(g - 1)
                back(NG - 1)

    nc.compile()
    return nc


def _band_mask(lo=None, hi=None):
    m = np.arange(2 * BL)[:, None]
    l = np.arange(BL)[None, :]
    f = (m - l >= 0) & (m - l < BL)
    if lo is not None:
        f = f & (m >= lo)
    if hi is not None:
        f = f & (m < hi)
    return f.astype(BF16)


def _stage(core, x, f, weights):
    s = core * LLOC
    xpad = np.zeros((C, XW), dtype=np.float32)
    a = max(0, s - (BL + HALF))
    b = min(L, s + LLOC + BL + HALF)
    xpad[:, a - (s - (BL + HALF)):b - (s - (BL + HALF))] = x[:, a:b]
    xbd = np.ascontiguousarray(
        xpad.reshape(2, P, XW).transpose(1, 0, 2)).astype(BF16)
    fpad = np.zeros((C, FW), dtype=np.float32)
    a = max(0, s - HALF)
    b = min(L, s - HALF + FW)
    fpad[:, a - (s - HALF):b - (s - HALF)] = f[:, a:b]
    fbd = np.ascontiguousarray(
        fpad.reshape(2, P, FW).transpose(1, 0, 2)).astype(FP8)

    band = _band_mask()
    interior = np.tile(band, (1, GB))
    first = interior.copy()
    if core == 0:
        first[:, 0:BL] = _band_mask(lo=HALF)
    last = interior.copy()
    if core == N_CORES - 1:
        last[:, (GB - 1) * BL:GB * BL] = _band_mask(hi=3 * HALF)
    fmv = np.concatenate([interior, first, last], axis=1).astype(BF16)

    m = {"xb": xbd, "fb": fbd, "fm": fmv,
         "xr": np.ascontiguousarray(x[:, s:s + LLOC]).astype(BF16),
         "ones": np.ones((P, P), dtype=BF16)}
    m.update(weights)
    return m


def _prep_weights(Wff, bff, Wq, bq, Wk, bk, Wv, bv, Wo, bo, Wc, bc):
    wff = np.zeros((P, 1536), dtype=BF16)
    for tap in range(3):
        for i in range(2):
            for o in range(2):
                blk = Wff[o * P:(o + 1) * P, i * P:(i + 1) * P, tap].T
                wff[:, ((tap * 2 + i) * 2 + o) * P:
                    ((tap * 2 + i) * 2 + o + 1) * P] = blk.astype(BF16)
    wqm = np.stack([Wq[:, h * P:(h + 1) * P].T for h in range(2)],
                   axis=1).astype(BF16)
    wkm = np.stack([Wk[:, h * P:(h + 1) * P].T for h in range(2)],
                   axis=1).astype(BF16)
    wvd = np.stack([Wv[:, h * P:(h + 1) * P].T for h in range(2)],
                   axis=1).astype(FP8)
    wom = np.concatenate(
        [Wo[o * P:(o + 1) * P, :].T for o in range(2)], axis=1).astype(BF16)
    wcm = np.zeros((P, 512), dtype=BF16)
    for i in range(2):
        for o in range(2):
            wcm[:, (i * 2 + o) * P:(i * 2 + o + 1) * P] = \
                Wc[o * P:(o + 1) * P, i * P:(i + 1) * P].T.astype(BF16)
    biasm = np.zeros((P, 9), dtype=np.float32)
    biasm[:, 0] = bff[:P]
    biasm[:, 1] = bff[P:]
    biasm[:, 2] = bq
    biasm[:, 3] = bk
    biasm[:, 4] = bv
    biasm[:, 5] = bo[:P]
    biasm[:, 6] = bo[P:]
    biasm[:, 7] = bc[:P]
    biasm[:, 8] = bc[P:]
    return {"wff": wff, "wq": wqm, "wk": wkm, "wvd": wvd, "wo": wom,
            "wc": wcm, "bias": biasm}


def kernel(x, f, mask, Wff, bff, Wq, bq, Wk, bk, Wv, bv, Wo, bo, Wc, bc,
           _trace=False, _trace_kwargs=None):
    x = np.asarray(x, dtype=np.float32)[0]
    f = np.asarray(f, dtype=np.float32)[0]
    weights = _prep_weights(
        np.asarray(Wff, np.float32), np.asarray(bff, np.float32),
        np.asarray(Wq, np.float32), np.asarray(bq, np.float32),
        np.asarray(Wk, np.float32), np.asarray(bk, np.float32),
        np.asarray(Wv, np.float32), np.asarray(bv, np.float32),
        np.asarray(Wo, np.float32), np.asarray(bo, np.float32),
        np.asarray(Wc, np.float32), np.asarray(bc, np.float32))

    if "nc" not in _CACHE:
        _CACHE["nc"] = _build_graph()
    nc = _CACHE["nc"]

    in_maps = [_stage(i, x, f, weights) for i in range(N_CORES)]
    res = run_bass_kernel_spmd(
        nc, in_maps, core_ids=list(range(N_CORES)),
        trace=_trace, **(_trace_kwargs or {}))
    y = np.concatenate(
        [np.asarray(res.results[i]["y"]) for i in range(N_CORES)], axis=1)
    out = y[None, :, :].astype(np.float32)
    if _trace:
        return out, res
    return out
